# revision 48
# baseline (speedup 1.0000x reference)
# Trainium2 Bass kernel for EndPointRepr (span endpoint representations).
#
# reference:
#   h = encoded_input @ W + b                    # [B, S, P]
#   res_k[q] = concat(h[qb[q], s_k[q]], h[qb[q], e_k[q]]) * (e_k[q] >= s_k[q])
#
# Sharding: data-parallel over batch. Core c owns batch c; the host routes
# each valid (e >= s) query to its batch's core. Invalid queries are never
# routed; the host-side result buffers start zeroed.
#
# Device pipeline (bf16 data path, fp32 PSUM accumulation):
#   The host compacts the batch to the h rows actually referenced (~1300 of
#   2048, capacity HROWS=1408) and remaps all four endpoint streams onto the
#   compacted row index space. All query-endpoint refs (stream, qid, row)
#   are sorted by compacted row and grouped by 128-row h block; each ref's
#   row lives in exactly one block, so the gather needs no accumulation.
#   phase 1: per 128-row block, 8 k-block matmuls (x-block stationary,
#            W streaming) accumulate h in PSUM; DVE folds the bias while
#            down-casting to bf16 SBUF tiles that stay resident. Identity
#            warmup matmuls ramp the PE p-state while the first DMAs land.
#   phase 2 (interleaved per block): gather AS MATMUL, transposed: the
#            128x128 h block half is the STATIONARY operand and a wide
#            host-built one-hot [128 rows, nqb] is the moving operand:
#            out[P-half, q] = h_block_half.T @ onehot. One matmul pair per
#            block (~2 x nqb cycles total) replaces the per-query-tile
#            window accumulation. DVE/ACT copy PSUM to bf16 staging tiles;
#            grouped DMAs write the transposed result; the host scatters
#            columns back to (query, endpoint) slots.
# bf16 keeps the PE at 1 cycle/row (fp32 is 4) and halves all DMA traffic;
# rel err ~3e-3 against the fp32 reference, well inside the 2e-2 gate.
import numpy as np

B, S, D, P = 8, 2048, 1024, 256
NQ = 8192
NCORES = 8
KB = D // 128          # contraction k-blocks
HB = 11                # compacted h row blocks
HROWS = HB * 128       # compacted h row capacity
NST = 4                # endpoint streams: s1, e1, s2, e2
NWARM = 26             # PE warmup matmuls (p-state ramp; bridges to first x)
OHSPLIT = 6            # oh loads in two pieces: blocks [0,OHSPLIT) early
QBMAX = 256            # per-block ref capacity (2*QBMAX fp32 = one PSUM bank)
GROUPS = [(0, 1, 2), (3, 4, 5), (6, 7, 8), (9,), (10,)]  # output DMA batching

_cache = {}


def _build_nc():
    import concourse.bacc as bacc
    import concourse.mybir as mybir
    import concourse.tile as tile

    f32 = mybir.dt.float32
    bf16 = mybir.dt.bfloat16
    nc = bacc.Bacc("TRN2", target_bir_lowering=False, debug=False,
                   num_devices=NCORES)

    nqb = _cache["nqb"]                    # per-block ref capacity (mult 4)
    bases = np.cumsum([0] + list(nqb)).tolist()
    nqtot = bases[-1]

    xh = nc.dram_tensor("xh", [128, HB * KB * 128], bf16,
                        kind="ExternalInput").ap()
    wh = nc.dram_tensor("wh", [128, KB * P], bf16, kind="ExternalInput").ap()
    # [128, 128] (512B/partition): col f<2 holds b[f*128+p]; rest is padding
    # so the DMA stays on the SDMA line-rate path (sub-512B descriptors go
    # through a slow RMW path and stall the whole queue behind them)
    bias = nc.dram_tensor("bias", [128, 128], f32, kind="ExternalInput").ap()
    oh = nc.dram_tensor("oh", [128, nqtot], bf16, kind="ExternalInput").ap()
    rt = nc.dram_tensor("rt", [128, 2 * nqtot], bf16,
                        kind="ExternalOutput").ap()
    osp = bases[min(OHSPLIT, HB)]

    with tile.TileContext(nc) as tc:
        with (
            tc.tile_pool(name="consts", bufs=1) as consts,
            tc.tile_pool(name="gout", bufs=len(GROUPS)) as g_pool,
            tc.tile_pool(name="ps", bufs=4, space="PSUM") as ps_pool,
            tc.tile_pool(name="psg", bufs=2, space="PSUM") as psg_pool,
            tc.tile_pool(name="psw", bufs=2, space="PSUM") as psw_pool,
        ):
            # Warmup must keep PE array duty near 100% so the HAM un-throttles
            # 3.4us in: alternate two stationary tiles (so LDWEIGHTS lands in
            # the background weight slot and overlaps the running matmul) and
            # stream a 256-wide moving operand.
            # N=256 moving operand: N=128 warmups leave the PE array duty
            # below the HAM busy threshold and the clock never ramps.
            warm_a = consts.tile([128, 128], bf16)
            warm_b = consts.tile([128, 128], bf16)
            warm_r = consts.tile([128, 256], bf16)
            nc.gpsimd.memset(warm_a, 0.0)
            nc.gpsimd.memset(warm_b, 0.0)
            nc.gpsimd.memset(warm_r, 0.0)
            for i in range(NWARM):
                warm_ps = psw_pool.tile([128, 256], f32, tag="warm")
                nc.tensor.matmul(warm_ps, warm_a if i % 2 == 0 else warm_b,
                                 warm_r, start=True, stop=True)

            # input DMAs: supply-ordered across the two HWDGE queues so each
            # tile lands (incl. ~1us completion receipt) before the PE needs
            # it; x chunks alternate queues so receipts pipeline.
            xt = consts.tile([128, HB, KB, 128], bf16)
            xh_view = xh.rearrange("p (b kb m) -> p b kb m", b=HB, kb=KB)
            w_sb = consts.tile([128, KB, P], bf16)
            bias_sb = consts.tile([128, 128], f32)
            oh_sb = consts.tile([128, nqtot], bf16)
            nc.sync.dma_start(w_sb,
                              wh.rearrange("p (kb j) -> p kb j", kb=KB))
            nc.scalar.dma_start(xt[:, 0:1], xh_view[:, 0:1])
            nc.sync.dma_start(xt[:, 1:2], xh_view[:, 1:2])
            nc.scalar.dma_start(oh_sb[:, 0:osp], oh[:, 0:osp])
            nc.sync.dma_start(xt[:, 2:3], xh_view[:, 2:3])
            nc.scalar.dma_start(xt[:, 3:4], xh_view[:, 3:4])
            nc.sync.dma_start(xt[:, 4:6], xh_view[:, 4:6])
            nc.scalar.dma_start(xt[:, 6:7], xh_view[:, 6:7])
            nc.scalar.dma_start(bias_sb, bias)
            if osp < nqtot:
                nc.scalar.dma_start(oh_sb[:, osp:nqtot], oh[:, osp:nqtot])
            nc.sync.dma_start(xt[:, 7:8], xh_view[:, 7:8])
            nc.sync.dma_start(xt[:, 8:9], xh_view[:, 8:9])
            nc.scalar.dma_start(xt[:, 9:10], xh_view[:, 9:10])
            nc.sync.dma_start(xt[:, 10:HB], xh_view[:, 10:HB])

            h_tiles = []
            stage = {"tile": None, "off": 0, "g": 0}

            def emit_gather(b):
                g = next(i for i, grp in enumerate(GROUPS) if b in grp)
                if b == GROUPS[g][0]:
                    stage["tile"] = g_pool.tile([128, 1600], bf16, tag="st",
                                                name=f"st{g}")
                    stage["off"] = 0
                    stage["g"] = g
                n = nqb[b]
                if n:
                    n2 = 2 * n
                    g_ps = psg_pool.tile([128, 512], f32, tag="gps",
                                         name=f"gps{b}")
                    for f in range(2):
                        nc.tensor.matmul(
                            g_ps[:, f * n:(f + 1) * n],
                            h_tiles[b][:, f * 128:(f + 1) * 128],
                            oh_sb[:, bases[b]:bases[b] + n],
                            start=True, stop=True)
                    off = stage["off"]
                    # copy + fold the (per-partition) bias per P-half;
                    # last block's copy + flush stay on the scalar engine so
                    # the tail chain avoids a cross-engine semaphore hop
                    for f in range(2):
                        dst = stage["tile"][:, off + f * n:off + (f + 1) * n]
                        src = g_ps[:, f * n:(f + 1) * n]
                        if b >= HB - 2 or b % 2 == 1:
                            nc.scalar.add(dst, src, bias_sb[:, f:f + 1])
                        else:
                            nc.vector.tensor_scalar_add(dst, src,
                                                        bias_sb[:, f:f + 1])
                    stage["off"] = off + n2
                if b == GROUPS[g][-1] and stage["off"]:
                    c0 = 2 * bases[GROUPS[g][0]]
                    eng = nc.scalar if b == HB - 1 else nc.sync
                    eng.dma_start(rt[:, c0:c0 + stage["off"]],
                                  stage["tile"][:, 0:stage["off"]])

            # phase 1 + interleaved phase 2
            for b in range(HB):
                h_ps = ps_pool.tile([128, P], f32, tag="hps", name=f"hps{b}")
                for kb in range(KB):
                    nc.tensor.matmul(
                        h_ps, xt[:, b, kb, :], w_sb[:, kb, :],
                        start=(kb == 0), stop=(kb == KB - 1))
                h_sb = consts.tile([128, P], bf16, name=f"h{b}")
                nc.vector.tensor_copy(h_sb, h_ps)
                h_tiles.append(h_sb)
                if b >= 1:
                    emit_gather(b - 1)
            emit_gather(HB - 1)

    nc.compile()
    return nc


def _get_nc(nqb):
    key = ("nc", tuple(nqb))
    if key not in _cache:
        _cache["nqb"] = list(nqb)
        _cache[key] = _build_nc()
    return _cache[key]


def _numpy_ref(flag, encoded_input, start_ids_1, end_ids_1, query_batch_idx,
               start_ids_2, end_ids_2, W, b):
    h = encoded_input.astype(np.float32) @ W.astype(np.float32) + \
        b.astype(np.float32)
    qb = np.asarray(query_batch_idx).astype(np.int64)

    def span(s, e):
        s = np.asarray(s).astype(np.int64)
        e = np.asarray(e).astype(np.int64)
        rep = np.concatenate([h[qb, s], h[qb, e]], axis=-1)
        return rep * (e >= s)[:, None].astype(rep.dtype)

    return span(start_ids_1, end_ids_1), span(start_ids_2, end_ids_2)


def kernel(flag, encoded_input, start_ids_1, end_ids_1, query_batch_idx,
           start_ids_2, end_ids_2, W, b):
    import ml_dtypes
    from concourse.bass_utils import run_bass_kernel_spmd

    bf16 = ml_dtypes.bfloat16
    x_full = np.asarray(encoded_input, dtype=np.float32)
    w_np = np.asarray(W, dtype=np.float32)
    b_np = np.asarray(b).astype(np.float32)
    qb = np.asarray(query_batch_idx).astype(np.int64)
    s1 = np.asarray(start_ids_1).astype(np.int64)
    e1 = np.asarray(end_ids_1).astype(np.int64)
    s2 = np.asarray(start_ids_2).astype(np.int64)
    e2 = np.asarray(end_ids_2).astype(np.int64)

    in_range = (qb.min() >= 0 and qb.max() < B and
                all(a.min() >= 0 and a.max() < S for a in (s1, e1, s2, e2)))

    percore = []
    try:
        if not in_range or x_full.shape != (B, S, D):
            raise ValueError("shape/range")
        for bb in range(B):
            sel = qb == bb
            ids1 = np.nonzero(sel & (e1 >= s1))[0]
            ids2 = np.nonzero(sel & (e2 >= s2))[0]
            rows = np.unique(np.concatenate(
                [s1[ids1], e1[ids1], s2[ids2], e2[ids2]]))
            if len(rows) > HROWS:
                raise ValueError("row overflow")
            # all query-endpoint refs (stream, qid, compacted row), row-sorted
            sts, qids, crs = [], [], []
            for st, (ids, a) in enumerate([(ids1, s1), (ids1, e1),
                                           (ids2, s2), (ids2, e2)]):
                sts.append(np.full(len(ids), st, np.int64))
                qids.append(ids)
                crs.append(np.searchsorted(rows, a[ids]).astype(np.int64))
            sts = np.concatenate(sts)
            qids = np.concatenate(qids)
            crs = np.concatenate(crs)
            o = np.argsort(crs, kind="stable")
            percore.append((rows, sts[o], qids[o], crs[o]))
        # per-block ref capacity: max count over cores, padded to mult of 4
        edges = np.arange(0, HROWS + 1, 128)
        counts = np.stack([np.searchsorted(pc[3], edges) for pc in percore])
        counts = counts[:, 1:] - counts[:, :-1]       # [B, HB]
        nqb = [int(-4 * (-counts[:, x].max() // 4)) for x in range(HB)]
        # pad the last block to >=128 cols so its solo output flush stays
        # above the 512B/partition SDMA line-rate threshold
        nqb[HB - 1] = max(nqb[HB - 1], 128)
        if max(nqb) > QBMAX:
            raise ValueError("block overflow")
        bases = np.cumsum([0] + nqb)
        nqtot = int(bases[-1])

        wh = np.ascontiguousarray(
            w_np.reshape(KB, 128, P).transpose(1, 0, 2).reshape(128, KB * P)
        ).astype(bf16)
        bias_rep = np.zeros((128, 128), np.float32)
        bias_rep[:, 0:2] = b_np.reshape(2, 128).T
        in_maps = []
        for bb in range(B):
            rows, sts, qids, crs = percore[bb]
            oh_np = np.zeros((128, nqtot), np.float32)
            be = np.searchsorted(crs, edges)
            for x in range(HB):
                seg = crs[be[x]:be[x + 1]]
                oh_np[seg - 128 * x, bases[x] + np.arange(len(seg))] = 1.0
            xc = np.zeros((HROWS, D), np.float32)
            xc[:len(rows)] = x_full[bb][rows]
            xr = xc.reshape(HB, 128, KB, 128).transpose(3, 0, 2, 1) \
                .reshape(128, HB * KB * 128)
            in_maps.append({
                "xh": np.ascontiguousarray(xr).astype(bf16),
                "wh": wh,
                "bias": bias_rep,
                "oh": np.ascontiguousarray(oh_np).astype(bf16),
            })
    except ValueError:
        res1, res2 = _numpy_ref(flag, x_full, s1, e1, qb, s2, e2, w_np, b_np)
        return np.asarray(res1, np.float32), np.asarray(res2, np.float32)

    nc = _get_nc(tuple(nqb))
    out = run_bass_kernel_spmd(nc, in_maps, core_ids=list(range(NCORES)))
    _cache["last_run"] = out

    res1 = np.zeros((NQ, 2 * P), np.float32)
    res2 = np.zeros((NQ, 2 * P), np.float32)
    for bb in range(B):
        rows, sts, qids, crs = percore[bb]
        rr = np.asarray(out.results[bb]["rt"]).astype(np.float32)
        be = np.searchsorted(crs, edges)
        for x in range(HB):
            lo, hi = be[x], be[x + 1]
            n = hi - lo
            if n == 0:
                continue
            j = np.arange(n)
            col0 = 2 * bases[x] + j
            col1 = 2 * bases[x] + nqb[x] + j
            st_b, qid_b = sts[lo:hi], qids[lo:hi]
            for res, stsel in [(res1, st_b < 2), (res2, st_b >= 2)]:
                for endp in range(2):
                    m = stsel & (st_b % 2 == endp)
                    if m.any():
                        res[qid_b[m], endp * P:endp * P + 128] = \
                            rr[:, col0[m]].T
                        res[qid_b[m], endp * P + 128:endp * P + 256] = \
                            rr[:, col1[m]].T
    return res1, res2


# revision 49
# speedup vs baseline: 1.1039x; 1.1039x over previous
# Trainium2 Bass kernel for EndPointRepr (span endpoint representations).
#
# reference:
#   h = encoded_input @ W + b                    # [B, S, P]
#   res_k[q] = concat(h[qb[q], s_k[q]], h[qb[q], e_k[q]]) * (e_k[q] >= s_k[q])
#
# Sharding: data-parallel over batch. Core c owns batch c; the host routes
# each valid (e >= s) query to its batch's core. Invalid queries are never
# routed; the host-side result buffers start zeroed.
#
# Device pipeline (bf16 data path, fp32 PSUM accumulation):
#   The host compacts the batch to the h rows actually referenced (~1300 of
#   2048, capacity HROWS=1408) and remaps all four endpoint streams onto the
#   compacted row index space. All query-endpoint refs (stream, qid, row)
#   are sorted by compacted row and grouped by 128-row h block; each ref's
#   row lives in exactly one block, so the gather needs no accumulation.
#   phase 1: per 128-row block, 8 k-block matmuls (x-block stationary,
#            W streaming) accumulate h in PSUM; DVE folds the bias while
#            down-casting to bf16 SBUF tiles that stay resident. Identity
#            warmup matmuls ramp the PE p-state while the first DMAs land.
#   phase 2 (interleaved per block): gather AS MATMUL, transposed: the
#            128x128 h block half is the STATIONARY operand and a wide
#            host-built one-hot [128 rows, nqb] is the moving operand:
#            out[P-half, q] = h_block_half.T @ onehot. One matmul pair per
#            block (~2 x nqb cycles total) replaces the per-query-tile
#            window accumulation. DVE/ACT copy PSUM to bf16 staging tiles;
#            grouped DMAs write the transposed result; the host scatters
#            columns back to (query, endpoint) slots.
# bf16 keeps the PE at 1 cycle/row (fp32 is 4) and halves all DMA traffic;
# rel err ~3e-3 against the fp32 reference, well inside the 2e-2 gate.
import numpy as np

B, S, D, P = 8, 2048, 1024, 256
NQ = 8192
NCORES = 8
KB = D // 128          # contraction k-blocks
HB = 11                # compacted h row blocks
HROWS = HB * 128       # compacted h row capacity
NST = 4                # endpoint streams: s1, e1, s2, e2
NWARM = 24             # PE warmup matmuls (p-state ramp; bridges to first x)
OHSPLIT = 6            # oh loads in two pieces: blocks [0,OHSPLIT) early
QBMAX = 256            # per-block ref capacity (2*QBMAX fp32 = one PSUM bank)
GROUPS = [(0, 1, 2), (3, 4, 5), (6, 7, 8), (9,), (10,)]  # output DMA batching

_cache = {}


def _build_nc():
    import concourse.bacc as bacc
    import concourse.mybir as mybir
    import concourse.tile as tile

    f32 = mybir.dt.float32
    bf16 = mybir.dt.bfloat16
    nc = bacc.Bacc("TRN2", target_bir_lowering=False, debug=False,
                   num_devices=NCORES)

    nqb = _cache["nqb"]                    # per-block ref capacity (mult 4)
    bases = np.cumsum([0] + list(nqb)).tolist()
    nqtot = bases[-1]

    xh = nc.dram_tensor("xh", [128, HB * KB * 128], bf16,
                        kind="ExternalInput").ap()
    wh = nc.dram_tensor("wh", [128, KB * P], bf16, kind="ExternalInput").ap()
    # [128, 128] (512B/partition): col f<2 holds b[f*128+p]; rest is padding
    # so the DMA stays on the SDMA line-rate path (sub-512B descriptors go
    # through a slow RMW path and stall the whole queue behind them)
    bias = nc.dram_tensor("bias", [128, 128], f32, kind="ExternalInput").ap()
    oh = nc.dram_tensor("oh", [128, nqtot], bf16, kind="ExternalInput").ap()
    rt = nc.dram_tensor("rt", [128, 2 * nqtot], bf16,
                        kind="ExternalOutput").ap()
    osp = bases[min(OHSPLIT, HB)]

    with tile.TileContext(nc) as tc:
        with (
            tc.tile_pool(name="consts", bufs=1) as consts,
            tc.tile_pool(name="gout", bufs=len(GROUPS)) as g_pool,
            tc.tile_pool(name="ps", bufs=4, space="PSUM") as ps_pool,
            tc.tile_pool(name="psg", bufs=2, space="PSUM") as psg_pool,
            tc.tile_pool(name="psw", bufs=2, space="PSUM") as psw_pool,
        ):
            # Warmup must keep PE array duty near 100% so the HAM un-throttles
            # 3.4us in: alternate two stationary tiles (so LDWEIGHTS lands in
            # the background weight slot and overlaps the running matmul) and
            # stream a 256-wide moving operand.
            # N=256 moving operand: N=128 warmups leave the PE array duty
            # below the HAM busy threshold and the clock never ramps.
            warm_a = consts.tile([128, 128], bf16)
            warm_b = consts.tile([128, 128], bf16)
            warm_r = consts.tile([128, 256], bf16)
            nc.gpsimd.memset(warm_a, 0.0)
            nc.gpsimd.memset(warm_b, 0.0)
            nc.gpsimd.memset(warm_r, 0.0)
            for i in range(NWARM):
                warm_ps = psw_pool.tile([128, 256], f32, tag="warm")
                nc.tensor.matmul(warm_ps, warm_a if i % 2 == 0 else warm_b,
                                 warm_r, start=True, stop=True)

            # input DMAs: supply-ordered across the two HWDGE queues so each
            # tile lands (incl. ~1us completion receipt) before the PE needs
            # it; x chunks alternate queues so receipts pipeline.
            xt = consts.tile([128, HB, KB, 128], bf16)
            xh_view = xh.rearrange("p (b kb m) -> p b kb m", b=HB, kb=KB)
            w_sb = consts.tile([128, KB, P], bf16)
            bias_sb = consts.tile([128, 128], f32)
            oh_sb = consts.tile([128, nqtot], bf16)
            nc.sync.dma_start(w_sb,
                              wh.rearrange("p (kb j) -> p kb j", kb=KB))
            nc.scalar.dma_start(xt[:, 0:1], xh_view[:, 0:1])
            nc.sync.dma_start(xt[:, 1:2], xh_view[:, 1:2])
            nc.scalar.dma_start(oh_sb[:, 0:osp], oh[:, 0:osp])
            nc.sync.dma_start(xt[:, 2:3], xh_view[:, 2:3])
            nc.scalar.dma_start(xt[:, 3:4], xh_view[:, 3:4])
            nc.scalar.dma_start(bias_sb, bias)
            nc.sync.dma_start(xt[:, 4:6], xh_view[:, 4:6])
            nc.scalar.dma_start(xt[:, 6:7], xh_view[:, 6:7])
            if osp < nqtot:
                nc.scalar.dma_start(oh_sb[:, osp:nqtot], oh[:, osp:nqtot])
            nc.sync.dma_start(xt[:, 7:8], xh_view[:, 7:8])
            nc.sync.dma_start(xt[:, 8:9], xh_view[:, 8:9])
            nc.scalar.dma_start(xt[:, 9:10], xh_view[:, 9:10])
            nc.sync.dma_start(xt[:, 10:HB], xh_view[:, 10:HB])

            h_tiles = []
            stage = {"tile": None, "off": 0, "g": 0}

            def emit_gather(b):
                g = next(i for i, grp in enumerate(GROUPS) if b in grp)
                if b == GROUPS[g][0]:
                    stage["tile"] = g_pool.tile([128, 1600], bf16, tag="st",
                                                name=f"st{g}")
                    stage["off"] = 0
                    stage["g"] = g
                n = nqb[b]
                if n:
                    n2 = 2 * n
                    g_ps = psg_pool.tile([128, 512], f32, tag="gps",
                                         name=f"gps{b}")
                    for f in range(2):
                        nc.tensor.matmul(
                            g_ps[:, f * n:(f + 1) * n],
                            h_tiles[b][:, f * 128:(f + 1) * 128],
                            oh_sb[:, bases[b]:bases[b] + n],
                            start=True, stop=True)
                    off = stage["off"]
                    # copy + fold the (per-partition) bias per P-half;
                    # last block's copy + flush stay on the scalar engine so
                    # the tail chain avoids a cross-engine semaphore hop
                    for f in range(2):
                        dst = stage["tile"][:, off + f * n:off + (f + 1) * n]
                        src = g_ps[:, f * n:(f + 1) * n]
                        if b >= HB - 2 or b % 2 == 1:
                            nc.scalar.add(dst, src, bias_sb[:, f:f + 1])
                        else:
                            nc.vector.tensor_scalar_add(dst, src,
                                                        bias_sb[:, f:f + 1])
                    stage["off"] = off + n2
                if b == GROUPS[g][-1] and stage["off"]:
                    c0 = 2 * bases[GROUPS[g][0]]
                    eng = nc.scalar if b == HB - 1 else nc.sync
                    eng.dma_start(rt[:, c0:c0 + stage["off"]],
                                  stage["tile"][:, 0:stage["off"]])

            # phase 1 + interleaved phase 2
            for b in range(HB):
                h_ps = ps_pool.tile([128, P], f32, tag="hps", name=f"hps{b}")
                for kb in range(KB):
                    nc.tensor.matmul(
                        h_ps, xt[:, b, kb, :], w_sb[:, kb, :],
                        start=(kb == 0), stop=(kb == KB - 1))
                h_sb = consts.tile([128, P], bf16, name=f"h{b}")
                nc.vector.tensor_copy(h_sb, h_ps)
                h_tiles.append(h_sb)
                if b >= 1:
                    emit_gather(b - 1)
            emit_gather(HB - 1)

    nc.compile()
    return nc


def _get_nc(nqb):
    key = ("nc", tuple(nqb))
    if key not in _cache:
        _cache["nqb"] = list(nqb)
        _cache[key] = _build_nc()
    return _cache[key]


def _numpy_ref(flag, encoded_input, start_ids_1, end_ids_1, query_batch_idx,
               start_ids_2, end_ids_2, W, b):
    h = encoded_input.astype(np.float32) @ W.astype(np.float32) + \
        b.astype(np.float32)
    qb = np.asarray(query_batch_idx).astype(np.int64)

    def span(s, e):
        s = np.asarray(s).astype(np.int64)
        e = np.asarray(e).astype(np.int64)
        rep = np.concatenate([h[qb, s], h[qb, e]], axis=-1)
        return rep * (e >= s)[:, None].astype(rep.dtype)

    return span(start_ids_1, end_ids_1), span(start_ids_2, end_ids_2)


def kernel(flag, encoded_input, start_ids_1, end_ids_1, query_batch_idx,
           start_ids_2, end_ids_2, W, b):
    import ml_dtypes
    from concourse.bass_utils import run_bass_kernel_spmd

    bf16 = ml_dtypes.bfloat16
    x_full = np.asarray(encoded_input, dtype=np.float32)
    w_np = np.asarray(W, dtype=np.float32)
    b_np = np.asarray(b).astype(np.float32)
    qb = np.asarray(query_batch_idx).astype(np.int64)
    s1 = np.asarray(start_ids_1).astype(np.int64)
    e1 = np.asarray(end_ids_1).astype(np.int64)
    s2 = np.asarray(start_ids_2).astype(np.int64)
    e2 = np.asarray(end_ids_2).astype(np.int64)

    in_range = (qb.min() >= 0 and qb.max() < B and
                all(a.min() >= 0 and a.max() < S for a in (s1, e1, s2, e2)))

    percore = []
    try:
        if not in_range or x_full.shape != (B, S, D):
            raise ValueError("shape/range")
        for bb in range(B):
            sel = qb == bb
            ids1 = np.nonzero(sel & (e1 >= s1))[0]
            ids2 = np.nonzero(sel & (e2 >= s2))[0]
            rows = np.unique(np.concatenate(
                [s1[ids1], e1[ids1], s2[ids2], e2[ids2]]))
            if len(rows) > HROWS:
                raise ValueError("row overflow")
            # all query-endpoint refs (stream, qid, compacted row), row-sorted
            sts, qids, crs = [], [], []
            for st, (ids, a) in enumerate([(ids1, s1), (ids1, e1),
                                           (ids2, s2), (ids2, e2)]):
                sts.append(np.full(len(ids), st, np.int64))
                qids.append(ids)
                crs.append(np.searchsorted(rows, a[ids]).astype(np.int64))
            sts = np.concatenate(sts)
            qids = np.concatenate(qids)
            crs = np.concatenate(crs)
            o = np.argsort(crs, kind="stable")
            percore.append((rows, sts[o], qids[o], crs[o]))
        # per-block ref capacity: max count over cores, padded to mult of 4
        edges = np.arange(0, HROWS + 1, 128)
        counts = np.stack([np.searchsorted(pc[3], edges) for pc in percore])
        counts = counts[:, 1:] - counts[:, :-1]       # [B, HB]
        nqb = [int(-4 * (-counts[:, x].max() // 4)) for x in range(HB)]
        # pad the last block to >=128 cols so its solo output flush stays
        # above the 512B/partition SDMA line-rate threshold
        nqb[HB - 1] = max(nqb[HB - 1], 128)
        if max(nqb) > QBMAX:
            raise ValueError("block overflow")
        bases = np.cumsum([0] + nqb)
        nqtot = int(bases[-1])

        wh = np.ascontiguousarray(
            w_np.reshape(KB, 128, P).transpose(1, 0, 2).reshape(128, KB * P)
        ).astype(bf16)
        bias_rep = np.zeros((128, 128), np.float32)
        bias_rep[:, 0:2] = b_np.reshape(2, 128).T
        in_maps = []
        for bb in range(B):
            rows, sts, qids, crs = percore[bb]
            oh_np = np.zeros((128, nqtot), np.float32)
            be = np.searchsorted(crs, edges)
            for x in range(HB):
                seg = crs[be[x]:be[x + 1]]
                oh_np[seg - 128 * x, bases[x] + np.arange(len(seg))] = 1.0
            xc = np.zeros((HROWS, D), np.float32)
            xc[:len(rows)] = x_full[bb][rows]
            xr = xc.reshape(HB, 128, KB, 128).transpose(3, 0, 2, 1) \
                .reshape(128, HB * KB * 128)
            in_maps.append({
                "xh": np.ascontiguousarray(xr).astype(bf16),
                "wh": wh,
                "bias": bias_rep,
                "oh": np.ascontiguousarray(oh_np).astype(bf16),
            })
    except ValueError:
        res1, res2 = _numpy_ref(flag, x_full, s1, e1, qb, s2, e2, w_np, b_np)
        return np.asarray(res1, np.float32), np.asarray(res2, np.float32)

    nc = _get_nc(tuple(nqb))
    out = run_bass_kernel_spmd(nc, in_maps, core_ids=list(range(NCORES)))
    _cache["last_run"] = out

    res1 = np.zeros((NQ, 2 * P), np.float32)
    res2 = np.zeros((NQ, 2 * P), np.float32)
    for bb in range(B):
        rows, sts, qids, crs = percore[bb]
        rr = np.asarray(out.results[bb]["rt"]).astype(np.float32)
        be = np.searchsorted(crs, edges)
        for x in range(HB):
            lo, hi = be[x], be[x + 1]
            n = hi - lo
            if n == 0:
                continue
            j = np.arange(n)
            col0 = 2 * bases[x] + j
            col1 = 2 * bases[x] + nqb[x] + j
            st_b, qid_b = sts[lo:hi], qids[lo:hi]
            for res, stsel in [(res1, st_b < 2), (res2, st_b >= 2)]:
                for endp in range(2):
                    m = stsel & (st_b % 2 == endp)
                    if m.any():
                        res[qid_b[m], endp * P:endp * P + 128] = \
                            rr[:, col0[m]].T
                        res[qid_b[m], endp * P + 128:endp * P + 256] = \
                            rr[:, col1[m]].T
    return res1, res2
